# revision 1
# baseline (speedup 1.0000x reference)
"""TRN2 Bass kernel v2 for nn_CropLayer (crop_and_resize, bilinear, 28x28).

Contract: kernel(images, boxes) takes the FULL inputs
  images [8, 512, 512, 32] f32, boxes [8, 100, 4] f32
and returns the FULL output [800, 28, 28, 32] f32, running on 8 NeuronCores
(data-parallel over batch, one image per core).

v2 design: the host repacks each image into two row-pair-interleaved copies
  P0[yp, x, r, c] = img[2*yp + r, x, c]          (even y0 pairs)
  P1[yp, x, r, c] = img[min(2*yp + 1 + r, 511), x, c]  (odd y0 pairs)
so the 2x2xC bilinear neighbourhood of any sample point is two 256 B units
at CONSECUTIVE unit addresses (unit = one x position = 2 rows x 32 ch).
Each point costs two 256 B gather descriptors (x0 unit and x1 unit) - half
the bytes of v1's two 512 B windows - and the blend needs only 4 weights:
  out = w_tl*X0.s0 + w_bl*X0.s1 + w_tr*X1.s0 + w_br*X1.s1   (7 DVE ops).

int16 gather indices cover one band of 64 yp values (64*512 units = 2^15);
points are host-sorted by (copy, band) into 8 classes, padded to /128, and
chunked; the host unsorts the device output.
"""

import sys

if '/opt/trn_rl_repo' not in sys.path:
    sys.path.insert(0, '/opt/trn_rl_repo')

import numpy as np

import concourse.bacc as bacc
import concourse.mybir as mybir
import concourse.tile as tile
import concourse.tile_sem_assignment as tsa
from concourse.ap import AP

F32 = mybir.dt.float32
I16 = mybir.dt.int16
OP = mybir.AluOpType

P = 128
H = 512
W = 512
C = 32
CROP = 28
NB = 100
PTS = CROP * CROP
NPT = NB * PTS
CHUNK = 6272              # points per chunk (= 128 * 49)
MCOL = CHUNK // P
NCLS = 8                  # (copy in {0,1}) x (band in {0..3})
YP_BAND = 64              # yp values per band; 64*512 = 32768 unit addrs
UNIT = 2 * C              # one x position: 2 rows x C channels (256 B)
COPY_ELEMS = (H // 2) * W * 2 * C   # 8.39 M f32 per repacked copy

NQ = 4                    # SWDGE queues
SINGLE_PACKET = False     # dma_gather packetization mode
MERGED = True             # one 512B desc per point (vs two 256B descs)
SORT_BY_ADDR = True       # order points by image address within a class

# ---------------------------------------------------------------------------
# Tile round-robins Pool-engine DMA insts over all 8 DMASW sem lanes with no
# regard for the SWDGE queue they run on, but each lane may only be updated
# from one queue (ucode constraint, enforced by CoreSim).  Patch the lane
# assignment so each queue gets a disjoint lane set.
_orig_assign_tick = tsa.TileClockTick._assign_tick
_IDXQ = {}


def _queue_aware_assign_tick(self, inst):
    qn = getattr(inst, "queue_num", None)
    if isinstance(inst, mybir.InstDMAGatherAnt) and qn is not None:
        ctr = _IDXQ.setdefault(id(self), {})
        c = ctr.get(qn, 0)
        ctr[qn] = c + 1
        step = 4 if NQ > 2 else 2
        # keep queues on disjoint lane sets for ANY lane count (loop
        # stages use 5 lanes instead of 8)
        n = self.swdge_sem_count
        slots = max(1, n // step)
        self.next_sw_dma_idx = (qn + step * (c % slots)) % n
    return _orig_assign_tick(self, inst)


if tsa.TileClockTick._assign_tick.__name__ != "_queue_aware_assign_tick":
    tsa.TileClockTick._assign_tick = _queue_aware_assign_tick


# ---------------------------------------------------------------------------
def _host_point_data(boxes_core):
    """Per-core box math in f32, mirroring the reference op-for-op.

    Returns, per (box, crop_y) unit of CROP points:
      cls   [NB, CROP]        class id = copy*4 + band
      idx0  [NB, CROP, CROP]  band-local unit index of x0 (int16)
      idx1  [NB, CROP, CROP]  band-local unit index of x1
      w4    [NB, CROP, CROP, 4]  folded blend weights (tl, bl, tr, br)
    """
    b = boxes_core.astype(np.float32)
    y1, x1, y2, x2 = b[:, 0], b[:, 1], b[:, 2], b[:, 3]
    g = np.arange(CROP, dtype=np.float32)
    hsc = (y2 - y1) * np.float32(H - 1) / np.float32(CROP - 1)
    wsc = (x2 - x1) * np.float32(W - 1) / np.float32(CROP - 1)
    in_y = y1[:, None] * np.float32(H - 1) + g[None, :] * hsc[:, None]
    in_x = x1[:, None] * np.float32(W - 1) + g[None, :] * wsc[:, None]

    vy = (in_y >= 0) & (in_y <= H - 1)
    vx = (in_x >= 0) & (in_x <= W - 1)
    y0f = np.floor(in_y)
    x0f = np.floor(in_x)
    ly = in_y - y0f
    lx = in_x - x0f
    y0 = np.clip(y0f.astype(np.int32), 0, H - 1)
    x0 = np.clip(x0f.astype(np.int32), 0, W - 1)
    x1i = np.minimum(x0 + 1, W - 1)

    copy = (y0 & 1).astype(np.int32)           # even y0 -> P0, odd -> P1
    yp = (y0 - copy) >> 1                      # row-pair index in its copy
    band = yp // YP_BAND
    ylocal = yp - band * YP_BAND
    cls = copy * 4 + band                      # [NB, CROP]

    base = (ylocal * W).astype(np.int32)       # [NB, CROP]
    idx0 = (base[:, :, None] + x0[:, None, :]).astype(np.int16)
    idx1 = (base[:, :, None] + x1i[:, None, :]).astype(np.int16)

    mask = (vy[:, :, None] & vx[:, None, :]).astype(np.float32)
    omly = (1 - ly)[:, :, None]
    omlx = (1 - lx)[:, None, :]
    w_tl = omly * omlx
    w_bl = ly[:, :, None] * omlx
    w_tr = omly * lx[:, None, :]
    w_br = ly[:, :, None] * lx[:, None, :]
    w4 = np.stack([w_tl, w_bl, w_tr, w_br], axis=-1) * mask[:, :, :, None]
    return cls, idx0, idx1, w4.astype(np.float32)


def _repack(img):
    """img [H, W, C] f32 -> (P0, P1) flat row-pair-interleaved copies."""
    p0 = np.ascontiguousarray(
        img.reshape(H // 2, 2, W, C).transpose(0, 2, 1, 3)).ravel()
    shifted = np.concatenate([img[1:], img[H - 1:]], axis=0)
    p1 = np.ascontiguousarray(
        shifted.reshape(H // 2, 2, W, C).transpose(0, 2, 1, 3)).ravel()
    return p0, p1


def _make_schedule(images, boxes):
    B = images.shape[0]
    per_core = []
    cls_counts = np.zeros((B, NCLS), np.int64)
    for c in range(B):
        cls, i0, i1, w4 = _host_point_data(boxes[c])
        per_core.append((cls, i0, i1, w4))
        for k in range(NCLS):
            cls_counts[c, k] = int((cls == k).sum()) * CROP

    pk = cls_counts.max(axis=0)
    pk = ((pk + P - 1) // P) * P
    total = int(pk.sum())
    L = ((total + CHUNK - 1) // CHUNK) * CHUNK
    last = int(np.nonzero(pk)[0][-1]) if pk.sum() else 0
    pk[last] += L - total

    segments = [[] for _ in range(L // CHUNK)]
    off = 0
    for k in range(NCLS):
        remaining = int(pk[k])
        while remaining > 0:
            ch = off // CHUNK
            room = CHUNK - (off % CHUNK)
            take = min(room, remaining)
            segments[ch].append((k, off % CHUNK, take))
            off += take
            remaining -= take

    in_maps = []
    unsort_rows = []
    for c in range(B):
        cls, i0, i1, w4 = per_core[c]
        # per-point flat views; point id = ((n*CROP + iy)*CROP + ix)
        cls_pt = np.repeat(cls.ravel(), CROP)          # [NPT]
        i0_pt = i0.reshape(-1)
        i1_pt = i1.reshape(-1)
        w4_pt = w4.reshape(-1, 4)
        flat0 = np.zeros(L, np.int16)
        flat1 = np.zeros(L, np.int16)
        flatW = np.zeros((L, 4), np.float32)
        pos_of = np.empty(NPT, np.int64)
        off = 0
        for k in range(NCLS):
            pts = np.nonzero(cls_pt == k)[0]
            if SORT_BY_ADDR and len(pts):
                pts = pts[np.argsort(i0_pt[pts], kind='stable')]
            npts = len(pts)
            if npts:
                sl = slice(off, off + npts)
                flat0[sl] = i0_pt[pts]
                flat1[sl] = i1_pt[pts]
                flatW[sl] = w4_pt[pts]
                pos_of[pts] = np.arange(off, off + npts)
            off += int(pk[k])
        wrap0 = flat0.reshape(L // 16, 16).T
        wrap1 = flat1.reshape(L // 16, 16).T
        idx0 = np.tile(wrap0, (8, 1)).copy()
        idx1 = np.tile(wrap1, (8, 1)).copy()
        wts = np.ascontiguousarray(
            flatW.reshape(L // P, P, 4).transpose(1, 0, 2).reshape(P, -1))
        p0, p1 = _repack(images[c])
        pad = np.zeros(UNIT, np.float32)   # MERGED reads 1 unit past the end
        p0 = np.concatenate([p0, pad])
        p1 = np.concatenate([p1, pad])
        in_maps.append({"p0": p0, "p1": p1, "idx0": idx0, "idx1": idx1,
                        "wts": wts})

        q = pos_of
        ch = q // CHUNK
        ql = q % CHUNK
        unsort_rows.append(ch * CHUNK + (ql % P) * MCOL + ql // P)

    return in_maps, segments, unsort_rows, L


def _build_nc(segments, L, num_devices=8, repeat=1):
    nc = bacc.Bacc("TRN2", target_bir_lowering=False, debug=False,
                   num_devices=num_devices, num_swdge_queues=NQ)
    p0_d = nc.dram_tensor("p0", [COPY_ELEMS + UNIT], F32,
                          kind="ExternalInput")
    p1_d = nc.dram_tensor("p1", [COPY_ELEMS + UNIT], F32,
                          kind="ExternalInput")
    idx0_d = nc.dram_tensor("idx0", [P, L // 16], I16, kind="ExternalInput")
    idx1_d = nc.dram_tensor("idx1", [P, L // 16], I16, kind="ExternalInput")
    wts_d = nc.dram_tensor("wts", [P, (L // P) * 4], F32, kind="ExternalInput")
    out_d = nc.dram_tensor("out", [L, C], F32, kind="ExternalOutput")

    nchunks = L // CHUNK
    outv = out_d.ap().rearrange("(c p q) e -> c p (q e)", c=nchunks, p=P)

    def class_in_ap(k, ew):
        copy, band = divmod(k, 4)
        src = p0_d if copy == 0 else p1_d
        return AP(src, band * YP_BAND * W * UNIT, [[UNIT, YP_BAND * W],
                                                   [1, ew]])

    with tile.TileContext(nc) as tc:
        with tc.tile_pool(name="persist", bufs=1) as pp:
            idx0 = pp.tile([P, L // 16], I16)
            idx1 = pp.tile([P, L // 16], I16)
            wts = pp.tile([P, (L // P) * 4], F32)
            nc.sync.dma_start(idx0[:], idx0_d.ap())
            nc.sync.dma_start(idx1[:], idx1_d.ap())
            nc.sync.dma_start(wts[:], wts_d.ap())
            wtsv = wts[:].rearrange("p (m s) -> p m s", s=4)

            with tc.tile_pool(name="work", bufs=2) as wp:

                for _ in range(repeat):
                    for ci in range(nchunks):
                        if MERGED:
                            X0 = wp.tile([P, CHUNK], F32, tag="X0")
                            G0 = X0[:].rearrange("p (m e) -> p m e",
                                                 e=2 * UNIT)
                            streams = ((idx0, G0),)
                            ew = 2 * UNIT
                        else:
                            X0 = wp.tile([P, CHUNK // 2], F32, tag="X0")
                            X1 = wp.tile([P, CHUNK // 2], F32, tag="X1")
                            G0 = X0[:].rearrange("p (m e) -> p m e", e=UNIT)
                            G1 = X1[:].rearrange("p (m e) -> p m e", e=UNIT)
                            streams = ((idx0, G0), (idx1, G1))
                            ew = UNIT
                        for (k, s0, cnt) in segments[ci]:
                            gcol = (ci * CHUNK + s0) // 16
                            m0 = s0 // P
                            mw = cnt // P
                            jobs = []
                            nsplit = NQ // len(streams)
                            for qn, (idx_sb, dst) in enumerate(streams):
                                if nsplit >= 2 and mw >= nsplit:
                                    mh = mw // nsplit
                                    off = 0
                                    for j in range(nsplit):
                                        mj = (mw - off if j == nsplit - 1
                                              else mh)
                                        jobs.append(
                                            (qn + j * len(streams), idx_sb,
                                             dst, m0 + off,
                                             mj, gcol + (off * P) // 16))
                                        off += mj
                                else:
                                    jobs.append((qn, idx_sb, dst, m0, mw,
                                                 gcol))
                            for (q, idx_sb, dst, mm0, mmw, gc) in jobs:
                                nc.gpsimd.dma_gather(
                                    out_ap=dst[:, mm0:mm0 + mmw, :],
                                    in_ap=class_in_ap(k, ew),
                                    idxs_ap=idx_sb[:, gc:gc
                                                   + (mmw * P) // 16],
                                    num_idxs=mmw * P,
                                    num_idxs_reg=mmw * P,
                                    elem_size=ew,
                                    elem_step=UNIT,
                                    single_packet=SINGLE_PACKET,
                                    queue_num=q % NQ,
                                )

                        res = wp.tile([P, MCOL * C], F32, tag="res")
                        tmp = wp.tile([P, MCOL * C], F32, tag="tmp")
                        r3 = res[:].rearrange("p (m e) -> p m e", e=C)
                        t3 = tmp[:].rearrange("p (m e) -> p m e", e=C)

                        def wb(s):
                            return (wtsv[:, ci * MCOL:(ci + 1) * MCOL,
                                         s:s + 1]
                                    .to_broadcast([P, MCOL, C]))

                        if MERGED:
                            srcs = [(G0, 0, 0), (G0, 1, 1), (G0, 2, 2),
                                    (G0, 3, 3)]
                        else:
                            srcs = [(G0, 0, 0), (G0, 1, 1), (G1, 0, 2),
                                    (G1, 1, 3)]
                        first = True
                        for (G3, half, s) in srcs:
                            sl = G3[:, :, half * C:half * C + C]
                            if first:
                                nc.vector.tensor_tensor(r3, sl, wb(s),
                                                        op=OP.mult)
                                first = False
                            else:
                                nc.vector.tensor_tensor(t3, sl, wb(s),
                                                        op=OP.mult)
                                nc.vector.tensor_tensor(r3, r3, t3,
                                                        op=OP.add)
                        nc.sync.dma_start(outv[ci], res[:])

    nc.compile()
    return nc


_NC_CACHE = {}


def kernel(images, boxes):
    images = np.ascontiguousarray(np.asarray(images, dtype=np.float32))
    boxes = np.ascontiguousarray(np.asarray(boxes, dtype=np.float32))
    B = images.shape[0]

    in_maps, segments, unsort_rows, L = _make_schedule(images, boxes)

    key = (B, L, tuple(tuple(s) for cs in segments for s in cs))
    nc = _NC_CACHE.get(key)
    if nc is None:
        nc = _build_nc(segments, L, num_devices=B)
        _NC_CACHE.clear()
        _NC_CACHE[key] = nc

    from concourse import bass_utils
    res = bass_utils.run_bass_kernel_spmd(nc, in_maps, core_ids=list(range(B)))

    outs = []
    for c in range(B):
        scratch = res.results[c]["out"]
        outs.append(scratch[unsort_rows[c]].reshape(NB, CROP, CROP, C))
    return np.concatenate(outs, axis=0)



# revision 5
# speedup vs baseline: 2.8621x; 2.8621x over previous
"""TRN2 Bass kernel v2 for nn_CropLayer (crop_and_resize, bilinear, 28x28).

Contract: kernel(images, boxes) takes the FULL inputs
  images [8, 512, 512, 32] f32, boxes [8, 100, 4] f32
and returns the FULL output [800, 28, 28, 32] f32, running on 8 NeuronCores
(data-parallel over batch, one image per core).

v2 design: the host repacks each image into two row-pair-interleaved copies
  P0[yp, x, r, c] = img[2*yp + r, x, c]          (even y0 pairs)
  P1[yp, x, r, c] = img[min(2*yp + 1 + r, 511), x, c]  (odd y0 pairs)
so the 2x2xC bilinear neighbourhood of any sample point is two 256 B units
at CONSECUTIVE unit addresses (unit = one x position = 2 rows x 32 ch).
Each point costs two 256 B gather descriptors (x0 unit and x1 unit) - half
the bytes of v1's two 512 B windows - and the blend needs only 4 weights:
  out = w_tl*X0.s0 + w_bl*X0.s1 + w_tr*X1.s0 + w_br*X1.s1   (7 DVE ops).

int16 gather indices cover one band of 64 yp values (64*512 units = 2^15);
points are host-sorted by (copy, band) into 8 classes, padded to /128, and
chunked; the host unsorts the device output.
"""

import sys

if '/opt/trn_rl_repo' not in sys.path:
    sys.path.insert(0, '/opt/trn_rl_repo')

import numpy as np

import concourse.bacc as bacc
import concourse.mybir as mybir
import concourse.tile as tile
import concourse.tile_sem_assignment as tsa
from concourse.ap import AP

F32 = mybir.dt.float32
I16 = mybir.dt.int16
OP = mybir.AluOpType

P = 128
H = 512
W = 512
C = 32
CROP = 28
NB = 100
PTS = CROP * CROP
NPT = NB * PTS
CHUNK = 6272              # points per chunk (= 128 * 49)
MCOL = CHUNK // P
NCLS = 8                  # (copy in {0,1}) x (band in {0..3})
YP_BAND = 64              # yp values per band; 64*512 = 32768 unit addrs
UNIT = 2 * C              # one x position: 2 rows x C channels (256 B)
COPY_ELEMS = (H // 2) * W * 2 * C   # 8.39 M f32 per repacked copy

NQ = 4                    # SWDGE queues
SINGLE_PACKET = False     # dma_gather packetization mode
MERGED = True             # one 512B desc per point (vs two 256B descs)
SORT_BY_ADDR = True       # order points by image address within a class

import os as _os
AB_SKIP_BLEND = _os.environ.get('AB_SKIP_BLEND', '0') == '1'
AB_SKIP_GATHER = _os.environ.get('AB_SKIP_GATHER', '0') == '1'
AB_SKIP_OUT = _os.environ.get('AB_SKIP_OUT', '0') == '1'

# ---------------------------------------------------------------------------
# Tile round-robins Pool-engine DMA insts over all 8 DMASW sem lanes with no
# regard for the SWDGE queue they run on, but each lane may only be updated
# from one queue (ucode constraint, enforced by CoreSim).  Patch the lane
# assignment so each queue gets a disjoint lane set.
if not hasattr(tsa, '_bass_orig_assign_tick'):
    tsa._bass_orig_assign_tick = tsa.TileClockTick._assign_tick
_orig_assign_tick = tsa._bass_orig_assign_tick
_IDXQ = {}


def _queue_aware_assign_tick(self, inst):
    qn = getattr(inst, "queue_num", None)
    if isinstance(inst, mybir.InstDMAGatherAnt) and qn is not None:
        ctr = _IDXQ.setdefault(id(self), {})
        c = ctr.get(qn, 0)
        ctr[qn] = c + 1
        step = 4 if NQ > 2 else 2
        # keep queues on disjoint lane sets for ANY lane count (loop
        # stages use 5 lanes instead of 8)
        n = self.swdge_sem_count
        slots = max(1, n // step)
        self.next_sw_dma_idx = (qn + step * (c % slots)) % n
    return _orig_assign_tick(self, inst)


if tsa.TileClockTick._assign_tick.__name__ != "_queue_aware_assign_tick":
    tsa.TileClockTick._assign_tick = _queue_aware_assign_tick


# ---------------------------------------------------------------------------
def _host_point_data(boxes_core):
    """Per-core box math in f32, mirroring the reference op-for-op.

    Returns, per (box, crop_y) unit of CROP points:
      cls   [NB, CROP]        class id = copy*4 + band
      idx0  [NB, CROP, CROP]  band-local unit index of x0 (int16)
      idx1  [NB, CROP, CROP]  band-local unit index of x1
      w4    [NB, CROP, CROP, 4]  folded blend weights (tl, bl, tr, br)
    """
    b = boxes_core.astype(np.float32)
    y1, x1, y2, x2 = b[:, 0], b[:, 1], b[:, 2], b[:, 3]
    g = np.arange(CROP, dtype=np.float32)
    hsc = (y2 - y1) * np.float32(H - 1) / np.float32(CROP - 1)
    wsc = (x2 - x1) * np.float32(W - 1) / np.float32(CROP - 1)
    in_y = y1[:, None] * np.float32(H - 1) + g[None, :] * hsc[:, None]
    in_x = x1[:, None] * np.float32(W - 1) + g[None, :] * wsc[:, None]

    vy = (in_y >= 0) & (in_y <= H - 1)
    vx = (in_x >= 0) & (in_x <= W - 1)
    y0f = np.floor(in_y)
    x0f = np.floor(in_x)
    ly = in_y - y0f
    lx = in_x - x0f
    y0 = np.clip(y0f.astype(np.int32), 0, H - 1)
    x0 = np.clip(x0f.astype(np.int32), 0, W - 1)
    x1i = np.minimum(x0 + 1, W - 1)

    copy = (y0 & 1).astype(np.int32)           # even y0 -> P0, odd -> P1
    yp = (y0 - copy) >> 1                      # row-pair index in its copy
    band = yp // YP_BAND
    ylocal = yp - band * YP_BAND
    cls = copy * 4 + band                      # [NB, CROP]

    base = (ylocal * W).astype(np.int32)       # [NB, CROP]
    idx0 = (base[:, :, None] + x0[:, None, :]).astype(np.int16)
    idx1 = (base[:, :, None] + x1i[:, None, :]).astype(np.int16)

    mask = (vy[:, :, None] & vx[:, None, :]).astype(np.float32)
    omly = (1 - ly)[:, :, None]
    omlx = (1 - lx)[:, None, :]
    w_tl = omly * omlx
    w_bl = ly[:, :, None] * omlx
    w_tr = omly * lx[:, None, :]
    w_br = ly[:, :, None] * lx[:, None, :]
    w4 = np.stack([w_tl, w_bl, w_tr, w_br], axis=-1) * mask[:, :, :, None]
    return cls, idx0, idx1, w4.astype(np.float32)


def _repack(img):
    """img [H, W, C] f32 -> (P0, P1) flat row-pair-interleaved copies."""
    p0 = np.ascontiguousarray(
        img.reshape(H // 2, 2, W, C).transpose(0, 2, 1, 3)).ravel()
    shifted = np.concatenate([img[1:], img[H - 1:]], axis=0)
    p1 = np.ascontiguousarray(
        shifted.reshape(H // 2, 2, W, C).transpose(0, 2, 1, 3)).ravel()
    return p0, p1


def _make_schedule(images, boxes):
    B = images.shape[0]
    per_core = []
    cls_counts = np.zeros((B, NCLS), np.int64)
    for c in range(B):
        cls, i0, i1, w4 = _host_point_data(boxes[c])
        per_core.append((cls, i0, i1, w4))
        for k in range(NCLS):
            cls_counts[c, k] = int((cls == k).sum()) * CROP

    pk = cls_counts.max(axis=0)
    pk = ((pk + P - 1) // P) * P
    total = int(pk.sum())
    L = ((total + CHUNK - 1) // CHUNK) * CHUNK
    last = int(np.nonzero(pk)[0][-1]) if pk.sum() else 0
    pk[last] += L - total

    segments = [[] for _ in range(L // CHUNK)]
    off = 0
    for k in range(NCLS):
        remaining = int(pk[k])
        while remaining > 0:
            ch = off // CHUNK
            room = CHUNK - (off % CHUNK)
            take = min(room, remaining)
            segments[ch].append((k, off % CHUNK, take))
            off += take
            remaining -= take

    in_maps = []
    unsort_rows = []
    for c in range(B):
        cls, i0, i1, w4 = per_core[c]
        # per-point flat views; point id = ((n*CROP + iy)*CROP + ix)
        cls_pt = np.repeat(cls.ravel(), CROP)          # [NPT]
        i0_pt = i0.reshape(-1)
        i1_pt = i1.reshape(-1)
        w4_pt = w4.reshape(-1, 4)
        flat0 = np.zeros(L, np.int16)
        flat1 = np.zeros(L, np.int16)
        flatW = np.zeros((L, 4), np.float32)
        pos_of = np.empty(NPT, np.int64)
        off = 0
        for k in range(NCLS):
            pts = np.nonzero(cls_pt == k)[0]
            if SORT_BY_ADDR and len(pts):
                pts = pts[np.argsort(i0_pt[pts], kind='stable')]
            npts = len(pts)
            if npts:
                sl = slice(off, off + npts)
                flat0[sl] = i0_pt[pts]
                flat1[sl] = i1_pt[pts]
                flatW[sl] = w4_pt[pts]
                pos_of[pts] = np.arange(off, off + npts)
            off += int(pk[k])
        wrap0 = flat0.reshape(L // 16, 16).T
        wrap1 = flat1.reshape(L // 16, 16).T
        idx0 = np.tile(wrap0, (8, 1)).copy()
        idx1 = np.tile(wrap1, (8, 1)).copy()
        wts = np.ascontiguousarray(
            flatW.reshape(L // P, P, 4).transpose(1, 0, 2).reshape(P, -1))
        p0, p1 = _repack(images[c])
        pad = np.zeros(UNIT, np.float32)   # MERGED reads 1 unit past the end
        p0 = np.concatenate([p0, pad])
        p1 = np.concatenate([p1, pad])
        in_maps.append({"p0": p0, "p1": p1, "idx0": idx0, "idx1": idx1,
                        "wts": wts})

        q = pos_of
        ch = q // CHUNK
        ql = q % CHUNK
        unsort_rows.append(ch * CHUNK + (ql % P) * MCOL + ql // P)

    return in_maps, segments, unsort_rows, L


def _build_nc(segments, L, num_devices=8, repeat=1):
    nc = bacc.Bacc("TRN2", target_bir_lowering=False, debug=False,
                   num_devices=num_devices, num_swdge_queues=NQ)
    p0_d = nc.dram_tensor("p0", [COPY_ELEMS + UNIT], F32,
                          kind="ExternalInput")
    p1_d = nc.dram_tensor("p1", [COPY_ELEMS + UNIT], F32,
                          kind="ExternalInput")
    idx0_d = nc.dram_tensor("idx0", [P, L // 16], I16, kind="ExternalInput")
    idx1_d = nc.dram_tensor("idx1", [P, L // 16], I16, kind="ExternalInput")
    wts_d = nc.dram_tensor("wts", [P, (L // P) * 4], F32, kind="ExternalInput")
    out_d = nc.dram_tensor("out", [L, C], F32, kind="ExternalOutput")

    nchunks = L // CHUNK
    outv = out_d.ap().rearrange("(c p q) e -> c p (q e)", c=nchunks, p=P)

    def class_in_ap(k, ew):
        copy, band = divmod(k, 4)
        src = p0_d if copy == 0 else p1_d
        return AP(src, band * YP_BAND * W * UNIT, [[UNIT, YP_BAND * W],
                                                   [1, ew]])

    with tile.TileContext(nc) as tc:
        with tc.tile_pool(name="persist", bufs=1) as pp:
            idx0 = pp.tile([P, L // 16], I16)
            idx1 = pp.tile([P, L // 16], I16)
            wts = pp.tile([P, (L // P) * 4], F32)
            nc.sync.dma_start(idx0[:], idx0_d.ap())
            nc.sync.dma_start(idx1[:], idx1_d.ap())
            nc.sync.dma_start(wts[:], wts_d.ap())
            wtsv = wts[:].rearrange("p (m s) -> p m s", s=4)

            with tc.tile_pool(name="work", bufs=2) as wp:

                for _ in range(repeat):
                    for ci in range(nchunks):
                        if MERGED:
                            X0 = wp.tile([P, CHUNK], F32, tag="X0")
                            G0 = X0[:].rearrange("p (m e) -> p m e",
                                                 e=2 * UNIT)
                            streams = ((idx0, G0),)
                            ew = 2 * UNIT
                        else:
                            X0 = wp.tile([P, CHUNK // 2], F32, tag="X0")
                            X1 = wp.tile([P, CHUNK // 2], F32, tag="X1")
                            G0 = X0[:].rearrange("p (m e) -> p m e", e=UNIT)
                            G1 = X1[:].rearrange("p (m e) -> p m e", e=UNIT)
                            streams = ((idx0, G0), (idx1, G1))
                            ew = UNIT
                        for (k, s0, cnt) in segments[ci]:
                            gcol = (ci * CHUNK + s0) // 16
                            m0 = s0 // P
                            mw = cnt // P
                            jobs = []
                            nsplit = NQ // len(streams)
                            for qn, (idx_sb, dst) in enumerate(streams):
                                if nsplit >= 2 and mw >= nsplit:
                                    mh = mw // nsplit
                                    off = 0
                                    for j in range(nsplit):
                                        mj = (mw - off if j == nsplit - 1
                                              else mh)
                                        jobs.append(
                                            (qn + j * len(streams), idx_sb,
                                             dst, m0 + off,
                                             mj, gcol + (off * P) // 16))
                                        off += mj
                                else:
                                    jobs.append((qn, idx_sb, dst, m0, mw,
                                                 gcol))
                            if AB_SKIP_GATHER:
                                jobs = []
                            for (q, idx_sb, dst, mm0, mmw, gc) in jobs:
                                nc.gpsimd.dma_gather(
                                    out_ap=dst[:, mm0:mm0 + mmw, :],
                                    in_ap=class_in_ap(k, ew),
                                    idxs_ap=idx_sb[:, gc:gc
                                                   + (mmw * P) // 16],
                                    num_idxs=mmw * P,
                                    num_idxs_reg=mmw * P,
                                    elem_size=ew,
                                    elem_step=UNIT,
                                    single_packet=SINGLE_PACKET,
                                    queue_num=q % NQ,
                                )

                        res = wp.tile([P, MCOL * C], F32, tag="res")
                        tmp = wp.tile([P, MCOL * C], F32, tag="tmp")
                        r3 = res[:].rearrange("p (m e) -> p m e", e=C)
                        t3 = tmp[:].rearrange("p (m e) -> p m e", e=C)

                        def wb(s):
                            return (wtsv[:, ci * MCOL:(ci + 1) * MCOL,
                                         s:s + 1]
                                    .to_broadcast([P, MCOL, C]))

                        if MERGED:
                            srcs = [(G0, 0, 0), (G0, 1, 1), (G0, 2, 2),
                                    (G0, 3, 3)]
                        else:
                            srcs = [(G0, 0, 0), (G0, 1, 1), (G1, 0, 2),
                                    (G1, 1, 3)]
                        first = True
                        if AB_SKIP_BLEND:
                            srcs = srcs[:1]
                        for (G3, half, s) in srcs:
                            sl = G3[:, :, half * C:half * C + C]
                            if first:
                                nc.vector.tensor_tensor(r3, sl, wb(s),
                                                        op=OP.mult)
                                first = False
                            else:
                                nc.vector.tensor_tensor(t3, sl, wb(s),
                                                        op=OP.mult)
                                nc.vector.tensor_tensor(r3, r3, t3,
                                                        op=OP.add)
                        if not AB_SKIP_OUT:
                            nc.sync.dma_start(outv[ci], res[:])

    nc.compile()
    return nc


_NC_CACHE = {}


def kernel(images, boxes):
    images = np.ascontiguousarray(np.asarray(images, dtype=np.float32))
    boxes = np.ascontiguousarray(np.asarray(boxes, dtype=np.float32))
    B = images.shape[0]

    in_maps, segments, unsort_rows, L = _make_schedule(images, boxes)

    key = (B, L, tuple(tuple(s) for cs in segments for s in cs))
    nc = _NC_CACHE.get(key)
    if nc is None:
        nc = _build_nc(segments, L, num_devices=B)
        _NC_CACHE.clear()
        _NC_CACHE[key] = nc

    from concourse import bass_utils
    res = bass_utils.run_bass_kernel_spmd(nc, in_maps, core_ids=list(range(B)))

    outs = []
    for c in range(B):
        scratch = res.results[c]["out"]
        outs.append(scratch[unsort_rows[c]].reshape(NB, CROP, CROP, C))
    return np.concatenate(outs, axis=0)



# revision 8
# speedup vs baseline: 3.3307x; 1.1638x over previous
"""TRN2 Bass kernel v5 for nn_CropLayer (crop_and_resize, bilinear, 28x28).

v5 = v3 (bf16 quadrant repack; one 256 B unit per sample point; DVE 2x-pair
blend) with the class scheme replaced by OVERLAPPING WINDOWS:

The 4 quadrant copies are concatenated into one DRAM tensor of 262144 units.
A gather instruction can address any compile-time 32768-unit window (int16
index limit).  Windows are placed every 16384 units (15 windows), so almost
every point has TWO candidate windows; each core balances its points between
them to meet shared per-window quotas, shrinking the SPMD max-over-cores
padding from ~11% to ~4%.  Pad slots use index 0 of their window (cheap
in-range fetch with zero blend weight).
"""

import sys

if '/opt/trn_rl_repo' not in sys.path:
    sys.path.insert(0, '/opt/trn_rl_repo')

import numpy as np

import concourse.bacc as bacc
import concourse.mybir as mybir
import concourse.tile as tile
import concourse.tile_sem_assignment as tsa
from concourse.ap import AP

F32 = mybir.dt.float32
BF16 = mybir.dt.bfloat16
I16 = mybir.dt.int16
OP = mybir.AluOpType
NPBF16 = mybir.dt.np(BF16)

P = 128
H = 512
W = 512
C = 32
CROP = 28
NB = 100
NPT = NB * CROP * CROP
CHUNK = 6272              # points per chunk (= 128 * 49)
MCOL = CHUNK // P
UNIT = 4 * C              # one unit: 2 rows x 2 x x 32 ch = 128 bf16 = 256 B
XP = W // 2               # 256 xp per row
QUNITS = (H // 2) * XP    # 65536 units per quadrant copy
WIN = 32768               # units addressable per gather (int16)
HWIN = WIN // 2           # window stride
NWIN = (4 * QUNITS - WIN) // HWIN + 1   # 15 windows over 262144 units

import os as _os

NQ = int(_os.environ.get('AB_NQ', '4'))   # SWDGE queues

AB_SKIP_BLEND = _os.environ.get('AB_SKIP_BLEND', '0') == '1'
AB_SKIP_GATHER = _os.environ.get('AB_SKIP_GATHER', '0') == '1'
AB_SKIP_OUT = _os.environ.get('AB_SKIP_OUT', '0') == '1'

# ---------------------------------------------------------------------------
# Tile round-robins Pool-engine DMA insts over all 8 DMASW sem lanes; keep
# each SWDGE queue on a disjoint lane set (ucode: one updater per lane).
if not hasattr(tsa, '_bass_orig_assign_tick'):
    tsa._bass_orig_assign_tick = tsa.TileClockTick._assign_tick
_orig_assign_tick = tsa._bass_orig_assign_tick
_IDXQ = {}


def _queue_aware_assign_tick(self, inst):
    qn = getattr(inst, "queue_num", None)
    if isinstance(inst, mybir.InstDMAGatherAnt) and qn is not None:
        ctr = _IDXQ.setdefault(id(self), {})
        c = ctr.get(qn, 0)
        ctr[qn] = c + 1
        step = 4 if NQ > 2 else 2
        n = self.swdge_sem_count
        slots = max(1, n // step)
        self.next_sw_dma_idx = (qn + step * (c % slots)) % n
    return _orig_assign_tick(self, inst)


if tsa.TileClockTick._assign_tick.__name__ != "_queue_aware_assign_tick":
    tsa.TileClockTick._assign_tick = _queue_aware_assign_tick


# ---------------------------------------------------------------------------
def _host_point_data(boxes_core):
    """Box math in f32 mirroring the reference. Returns per-point global
    unit index (int32, over the 4-copy concatenated layout) and bf16
    pair-duplicated blend weights."""
    b = boxes_core.astype(np.float32)
    y1, x1, y2, x2 = b[:, 0], b[:, 1], b[:, 2], b[:, 3]
    g = np.arange(CROP, dtype=np.float32)
    hsc = (y2 - y1) * np.float32(H - 1) / np.float32(CROP - 1)
    wsc = (x2 - x1) * np.float32(W - 1) / np.float32(CROP - 1)
    in_y = y1[:, None] * np.float32(H - 1) + g[None, :] * hsc[:, None]
    in_x = x1[:, None] * np.float32(W - 1) + g[None, :] * wsc[:, None]

    vy = (in_y >= 0) & (in_y <= H - 1)
    vx = (in_x >= 0) & (in_x <= W - 1)
    y0f = np.floor(in_y)
    x0f = np.floor(in_x)
    ly = in_y - y0f
    lx = in_x - x0f
    y0 = np.clip(y0f.astype(np.int32), 0, H - 1)
    x0 = np.clip(x0f.astype(np.int32), 0, W - 1)

    ry = y0 & 1
    rx = x0 & 1
    yp = y0 >> 1
    xp = x0 >> 1

    quad = ry[:, :, None] * 2 + rx[:, None, :]            # [NB, cy, cx]
    gu = (quad * QUNITS + yp[:, :, None] * XP + xp[:, None, :]).astype(
        np.int32)

    mask = (vy[:, :, None] & vx[:, None, :]).astype(np.float32)
    omly = (1 - ly)[:, :, None]
    omlx = (1 - lx)[:, None, :]
    w_tl = omly * omlx
    w_bl = ly[:, :, None] * omlx
    w_tr = omly * lx[:, None, :]
    w_br = ly[:, :, None] * lx[:, None, :]
    # unit layout [r, s, c] -> corner order tl, tr, bl, br
    w4 = np.stack([w_tl, w_tr, w_bl, w_br], axis=-1) * mask[:, :, :, None]
    w8 = np.repeat(w4.astype(NPBF16), 2, axis=-1)
    return gu.ravel(), w8.reshape(-1, 8)


def _repack4(img):
    """img [H, W, C] f32 -> one flat bf16 tensor of 4 quadrant copies."""
    parts = []
    img = img.astype(NPBF16)
    for ry in (0, 1):
        a = np.concatenate([img[ry:], img[H - 1:]], axis=0)[:H] if ry else img
        for rx in (0, 1):
            b = (np.concatenate([a[:, rx:], a[:, W - 1:]], axis=1)[:, :W]
                 if rx else a)
            t = b.reshape(H // 2, 2, XP, 2, C).transpose(0, 2, 1, 3, 4)
            parts.append(np.ascontiguousarray(t).ravel())
    return np.concatenate(parts)


def _window_quotas(halves_counts):
    """halves_counts [B, NH]: per-core point counts per HWIN-sized half.
    Window w covers halves {w, w+1}.  Compute shared quotas Q[w] (multiples
    of P) so every core's points fit greedily left-to-right."""
    B, NH = halves_counts.shape
    pref = np.cumsum(halves_counts, axis=1)      # Pm(c, t) = sum h<=t
    Q = np.zeros(NWIN, np.int64)
    acc = 0                                       # sum of Q[0..w-1]
    for w in range(NWIN):
        # after windows 0..w, all points of halves 0..w must be placed
        need = int(pref[:, w].max()) - acc
        q = max(0, need)
        q = -(-q // P) * P
        # last window must also hold the final half
        if w == NWIN - 1:
            need2 = int(pref[:, NH - 1].max()) - acc
            q = max(q, -(-max(0, need2) // P) * P)
        Q[w] = q
        acc += q
    return Q


def _assign(gu_sorted, Q):
    """Greedy left-to-right assignment of sorted points to windows.
    Returns per-window slices (start, count) into the sorted order."""
    n = len(gu_sorted)
    h = gu_sorted >> 14                           # half id (0..NWIN)
    counts = np.bincount(h, minlength=NWIN + 1)
    starts = np.zeros(NWIN + 2, np.int64)
    starts[1:] = np.cumsum(counts)
    # walk halves; window w takes tail of half w (after w-1 took some) plus
    # head of half w+1
    taken_from_half = np.zeros(NWIN + 1, np.int64)
    win_n = np.zeros(NWIN, np.int64)
    assign_w = np.empty(n, np.int64)
    for w in range(NWIN):
        cap = Q[w]
        # first take what's left of half w
        avail_w = counts[w] - taken_from_half[w]
        t0 = min(avail_w, cap)
        if t0 > 0:
            s = starts[w] + taken_from_half[w]
            assign_w[s:s + t0] = w
            taken_from_half[w] += t0
            cap -= t0
        # then head of half w+1
        if w + 1 <= NWIN:
            avail_n = counts[w + 1] - taken_from_half[w + 1]
            t1 = min(avail_n, cap)
            if t1 > 0:
                s = starts[w + 1] + taken_from_half[w + 1]
                assign_w[s:s + t1] = w
                taken_from_half[w + 1] += t1
                cap -= t1
        win_n[w] = Q[w] - cap
    assert taken_from_half[:NWIN + 1].sum() == n, "assignment incomplete"
    return assign_w, win_n


def _make_schedule(images, boxes):
    B = images.shape[0]
    per_core = []
    hc = np.zeros((B, NWIN + 1), np.int64)
    for c in range(B):
        gu, w8 = _host_point_data(boxes[c])
        order = np.argsort(gu, kind='stable')
        gus = gu[order]
        per_core.append((gus, order, w8))
        hcounts = np.bincount(gus >> 14, minlength=NWIN + 1)
        hc[c] = hcounts

    Q = _window_quotas(hc)
    pk = Q                                        # idx slots per window
    total = int(pk.sum())
    L = ((total + CHUNK - 1) // CHUNK) * CHUNK
    last = int(np.nonzero(pk)[0][-1])
    pk = pk.copy()
    pk[last] += L - total

    segments = [[] for _ in range(L // CHUNK)]
    off = 0
    for k in range(NWIN):
        remaining = int(pk[k])
        while remaining > 0:
            ch = off // CHUNK
            room = CHUNK - (off % CHUNK)
            take = min(room, remaining)
            segments[ch].append((k, off % CHUNK, take))
            off += take
            remaining -= take

    in_maps = []
    unsort_rows = []
    base_of = np.cumsum(np.concatenate([[0], pk[:-1]]))
    for c in range(B):
        gus, order, w8 = per_core[c]
        assign_w, win_n = _assign(gus, Q)
        flat = np.zeros(L, np.int16)
        flatW = np.zeros((L, 8), NPBF16)
        pos_of = np.empty(NPT, np.int64)
        # place points window by window (they are contiguous in sorted order
        # except interleaving at half boundaries; gather positions per window)
        for k in range(NWIN):
            pts_sorted_idx = np.nonzero(assign_w == k)[0]
            npts = len(pts_sorted_idx)
            assert npts <= pk[k], (k, npts, pk[k])
            sl = slice(int(base_of[k]), int(base_of[k]) + npts)
            rel = gus[pts_sorted_idx] - k * HWIN
            assert (rel >= 0).all() and (rel < WIN).all()
            flat[sl] = rel.astype(np.int16)
            pid = order[pts_sorted_idx]
            flatW[sl] = w8[pid]
            pos_of[pid] = np.arange(sl.start, sl.start + npts)

        wrap = flat.reshape(L // 16, 16).T
        idxw = np.tile(wrap, (8, 1)).copy()
        wts = np.ascontiguousarray(
            flatW.reshape(L // P, P, 8).transpose(1, 0, 2).reshape(P, -1))
        cp = _repack4(images[c])
        in_maps.append({"cp": cp, "idx": idxw, "wts": wts})

        q = pos_of
        ch = q // CHUNK
        ql = q % CHUNK
        unsort_rows.append(ch * CHUNK + (ql % P) * MCOL + ql // P)

    return in_maps, segments, unsort_rows, L


def _build_nc(segments, L, num_devices=8, repeat=1):
    nc = bacc.Bacc("TRN2", target_bir_lowering=False, debug=False,
                   num_devices=num_devices, num_swdge_queues=NQ)
    cp_d = nc.dram_tensor("cp", [4 * QUNITS * UNIT], BF16,
                          kind="ExternalInput")
    idx_d = nc.dram_tensor("idx", [P, L // 16], I16, kind="ExternalInput")
    wts_d = nc.dram_tensor("wts", [P, (L // P) * 8], BF16,
                           kind="ExternalInput")
    out_d = nc.dram_tensor("out", [L, C], BF16, kind="ExternalOutput")

    nchunks = L // CHUNK
    outv = out_d.ap().rearrange("(c p q) e -> c p (q e)", c=nchunks, p=P)

    def win_in_ap(k):
        return AP(cp_d, k * HWIN * UNIT, [[UNIT, WIN], [1, UNIT]])

    with tile.TileContext(nc) as tc:
        with tc.tile_pool(name="persist", bufs=1) as pp:
            idx = pp.tile([P, L // 16], I16)
            wts = pp.tile([P, (L // P) * 8], BF16)
            nc.sync.dma_start(idx[:], idx_d.ap())
            nc.sync.dma_start(wts[:], wts_d.ap())
            wtsv = wts[:].rearrange("p (m s) -> p m s", s=8)

            with tc.tile_pool(name="work", bufs=2) as wp:

                for _ in range(repeat):
                    for ci in range(nchunks):
                        X0 = wp.tile([P, CHUNK], BF16, tag="X0")
                        G0 = X0[:].rearrange("p (m e) -> p m e", e=UNIT)
                        if not AB_SKIP_GATHER:
                            for (k, s0, cnt) in segments[ci]:
                                gcol = (ci * CHUNK + s0) // 16
                                m0 = s0 // P
                                mw = cnt // P
                                jobs = []
                                if mw >= NQ:
                                    mh = mw // NQ
                                    off = 0
                                    for j in range(NQ):
                                        mj = (mw - off if j == NQ - 1
                                              else mh)
                                        jobs.append(
                                            (j, m0 + off, mj,
                                             gcol + (off * P) // 16))
                                        off += mj
                                else:
                                    for j in range(mw):
                                        jobs.append((j % NQ, m0 + j, 1,
                                                     gcol + (j * P) // 16))
                                for (q, mm0, mmw, gc) in jobs:
                                    nc.gpsimd.dma_gather(
                                        out_ap=G0[:, mm0:mm0 + mmw, :],
                                        in_ap=win_in_ap(k),
                                        idxs_ap=idx[:, gc:gc
                                                    + (mmw * P) // 16],
                                        num_idxs=mmw * P,
                                        num_idxs_reg=mmw * P,
                                        elem_size=UNIT,
                                        elem_step=UNIT,
                                        single_packet=False,
                                        queue_num=q,
                                    )

                        res = wp.tile([P, MCOL * C], BF16, tag="res")
                        tmp = wp.tile([P, MCOL * C], BF16, tag="tmp")
                        r4 = res[:].rearrange("p (m c2 two) -> p m c2 two",
                                              m=MCOL, two=2)
                        t4 = tmp[:].rearrange("p (m c2 two) -> p m c2 two",
                                              m=MCOL, two=2)

                        def wb(s):
                            return (wtsv[:, ci * MCOL:(ci + 1) * MCOL,
                                         2 * s:2 * s + 2]
                                    .unsqueeze(2)
                                    .to_broadcast([P, MCOL, C // 2, 2]))

                        srcs = list(range(4))
                        if AB_SKIP_BLEND:
                            srcs = srcs[:1]
                        first = True
                        for s in srcs:
                            sl = (G0[:, :, s * C:(s + 1) * C]
                                  .rearrange("p m (c2 two) -> p m c2 two",
                                             two=2))
                            if first:
                                nc.vector.tensor_tensor(r4, sl, wb(s),
                                                        op=OP.mult)
                                first = False
                            else:
                                nc.vector.tensor_tensor(t4, sl, wb(s),
                                                        op=OP.mult)
                                nc.vector.tensor_tensor(r4, r4, t4,
                                                        op=OP.add)
                        if not AB_SKIP_OUT:
                            nc.sync.dma_start(outv[ci], res[:])

    nc.compile()
    return nc


_NC_CACHE = {}


def kernel(images, boxes):
    images = np.ascontiguousarray(np.asarray(images, dtype=np.float32))
    boxes = np.ascontiguousarray(np.asarray(boxes, dtype=np.float32))
    B = images.shape[0]

    in_maps, segments, unsort_rows, L = _make_schedule(images, boxes)

    key = (B, L, tuple(tuple(s) for cs in segments for s in cs))
    nc = _NC_CACHE.get(key)
    if nc is None:
        nc = _build_nc(segments, L, num_devices=B)
        _NC_CACHE.clear()
        _NC_CACHE[key] = nc

    from concourse import bass_utils
    res = bass_utils.run_bass_kernel_spmd(nc, in_maps, core_ids=list(range(B)))

    outs = []
    for c in range(B):
        scratch = res.results[c]["out"]
        outs.append(scratch[unsort_rows[c]].astype(np.float32)
                    .reshape(NB, CROP, CROP, C))
    return np.concatenate(outs, axis=0)


# revision 9
# speedup vs baseline: 4.1520x; 1.2466x over previous
"""TRN2 Bass kernel v5 for nn_CropLayer (crop_and_resize, bilinear, 28x28).

v5 = v3 (bf16 quadrant repack; one 256 B unit per sample point; DVE 2x-pair
blend) with the class scheme replaced by OVERLAPPING WINDOWS:

The 4 quadrant copies are concatenated into one DRAM tensor of 262144 units.
A gather instruction can address any compile-time 32768-unit window (int16
index limit).  Windows are placed every 16384 units (15 windows), so almost
every point has TWO candidate windows; each core balances its points between
them to meet shared per-window quotas, shrinking the SPMD max-over-cores
padding from ~11% to ~4%.  Pad slots use index 0 of their window (cheap
in-range fetch with zero blend weight).
"""

import sys

if '/opt/trn_rl_repo' not in sys.path:
    sys.path.insert(0, '/opt/trn_rl_repo')

import numpy as np

import concourse.bacc as bacc
import concourse.mybir as mybir
import concourse.tile as tile
import concourse.tile_sem_assignment as tsa
from concourse.ap import AP

F32 = mybir.dt.float32
BF16 = mybir.dt.bfloat16
I16 = mybir.dt.int16
OP = mybir.AluOpType
NPBF16 = mybir.dt.np(BF16)

P = 128
H = 512
W = 512
C = 32
CROP = 28
NB = 100
NPT = NB * CROP * CROP
CHUNK = 6272              # points per chunk (= 128 * 49)
MCOL = CHUNK // P
UNIT = 4 * C              # one unit: 2 rows x 2 x x 32 ch = 128 bf16 = 256 B
XP = W // 2               # 256 xp per row
QUNITS = (H // 2) * XP    # 65536 units per quadrant copy
WIN = 32768               # units addressable per gather (int16)
HWIN = WIN // 2           # window stride
NWIN = (4 * QUNITS - WIN) // HWIN + 1   # 15 windows over 262144 units

import os as _os

NQ = int(_os.environ.get('AB_NQ', '4'))   # SWDGE queues

AB_SKIP_BLEND = _os.environ.get('AB_SKIP_BLEND', '0') == '1'
AB_SKIP_GATHER = _os.environ.get('AB_SKIP_GATHER', '0') == '1'
AB_SKIP_OUT = _os.environ.get('AB_SKIP_OUT', '0') == '1'

# ---------------------------------------------------------------------------
# Tile round-robins Pool-engine DMA insts over all 8 DMASW sem lanes; keep
# each SWDGE queue on a disjoint lane set (ucode: one updater per lane).
if not hasattr(tsa, '_bass_orig_assign_tick'):
    tsa._bass_orig_assign_tick = tsa.TileClockTick._assign_tick
_orig_assign_tick = tsa._bass_orig_assign_tick
_IDXQ = {}


def _queue_aware_assign_tick(self, inst):
    qn = getattr(inst, "queue_num", None)
    if isinstance(inst, mybir.InstDMAGatherAnt) and qn is not None:
        ctr = _IDXQ.setdefault(id(self), {})
        c = ctr.get(qn, 0)
        ctr[qn] = c + 1
        step = 4 if NQ > 2 else 2
        n = self.swdge_sem_count
        slots = max(1, n // step)
        self.next_sw_dma_idx = (qn + step * (c % slots)) % n
    return _orig_assign_tick(self, inst)


if tsa.TileClockTick._assign_tick.__name__ != "_queue_aware_assign_tick":
    tsa.TileClockTick._assign_tick = _queue_aware_assign_tick


# ---------------------------------------------------------------------------
def _host_point_data(boxes_core):
    """Box math in f32 mirroring the reference. Returns per-point global
    unit index (int32, over the 4-copy concatenated layout) and bf16
    pair-duplicated blend weights."""
    b = boxes_core.astype(np.float32)
    y1, x1, y2, x2 = b[:, 0], b[:, 1], b[:, 2], b[:, 3]
    g = np.arange(CROP, dtype=np.float32)
    hsc = (y2 - y1) * np.float32(H - 1) / np.float32(CROP - 1)
    wsc = (x2 - x1) * np.float32(W - 1) / np.float32(CROP - 1)
    in_y = y1[:, None] * np.float32(H - 1) + g[None, :] * hsc[:, None]
    in_x = x1[:, None] * np.float32(W - 1) + g[None, :] * wsc[:, None]

    vy = (in_y >= 0) & (in_y <= H - 1)
    vx = (in_x >= 0) & (in_x <= W - 1)
    y0f = np.floor(in_y)
    x0f = np.floor(in_x)
    ly = in_y - y0f
    lx = in_x - x0f
    y0 = np.clip(y0f.astype(np.int32), 0, H - 1)
    x0 = np.clip(x0f.astype(np.int32), 0, W - 1)

    ry = y0 & 1
    rx = x0 & 1
    yp = y0 >> 1
    xp = x0 >> 1

    quad = ry[:, :, None] * 2 + rx[:, None, :]            # [NB, cy, cx]
    gu = (quad * QUNITS + yp[:, :, None] * XP + xp[:, None, :]).astype(
        np.int32)

    mask = (vy[:, :, None] & vx[:, None, :]).astype(np.float32)
    omly = (1 - ly)[:, :, None]
    omlx = (1 - lx)[:, None, :]
    w_tl = omly * omlx
    w_bl = ly[:, :, None] * omlx
    w_tr = omly * lx[:, None, :]
    w_br = ly[:, :, None] * lx[:, None, :]
    # unit layout [r, s, c] -> corner order tl, tr, bl, br
    w4 = np.stack([w_tl, w_tr, w_bl, w_br], axis=-1) * mask[:, :, :, None]
    w8 = np.repeat(w4.astype(NPBF16), 2, axis=-1)
    return gu.ravel(), w8.reshape(-1, 8)


def _repack4(img):
    """img [H, W, C] f32 -> one flat bf16 tensor of 4 quadrant copies."""
    parts = []
    img = img.astype(NPBF16)
    for ry in (0, 1):
        a = np.concatenate([img[ry:], img[H - 1:]], axis=0)[:H] if ry else img
        for rx in (0, 1):
            b = (np.concatenate([a[:, rx:], a[:, W - 1:]], axis=1)[:, :W]
                 if rx else a)
            t = b.reshape(H // 2, 2, XP, 2, C).transpose(0, 2, 1, 3, 4)
            parts.append(np.ascontiguousarray(t).ravel())
    return np.concatenate(parts)


def _window_quotas(halves_counts):
    """halves_counts [B, NH]: per-core point counts per HWIN-sized half.
    Window w covers halves {w, w+1}.  Compute shared quotas Q[w] (multiples
    of P) so every core's points fit greedily left-to-right."""
    B, NH = halves_counts.shape
    pref = np.cumsum(halves_counts, axis=1)      # Pm(c, t) = sum h<=t
    Q = np.zeros(NWIN, np.int64)
    acc = 0                                       # sum of Q[0..w-1]
    for w in range(NWIN):
        # after windows 0..w, all points of halves 0..w must be placed
        need = int(pref[:, w].max()) - acc
        q = max(0, need)
        q = -(-q // P) * P
        # last window must also hold the final half
        if w == NWIN - 1:
            need2 = int(pref[:, NH - 1].max()) - acc
            q = max(q, -(-max(0, need2) // P) * P)
        Q[w] = q
        acc += q
    return Q


def _assign(gu_sorted, Q):
    """Greedy left-to-right assignment of sorted points to windows.
    Returns per-window slices (start, count) into the sorted order."""
    n = len(gu_sorted)
    h = gu_sorted >> 14                           # half id (0..NWIN)
    counts = np.bincount(h, minlength=NWIN + 1)
    starts = np.zeros(NWIN + 2, np.int64)
    starts[1:] = np.cumsum(counts)
    # walk halves; window w takes tail of half w (after w-1 took some) plus
    # head of half w+1
    taken_from_half = np.zeros(NWIN + 1, np.int64)
    win_n = np.zeros(NWIN, np.int64)
    assign_w = np.empty(n, np.int64)
    for w in range(NWIN):
        cap = Q[w]
        # first take what's left of half w
        avail_w = counts[w] - taken_from_half[w]
        t0 = min(avail_w, cap)
        if t0 > 0:
            s = starts[w] + taken_from_half[w]
            assign_w[s:s + t0] = w
            taken_from_half[w] += t0
            cap -= t0
        # then head of half w+1
        if w + 1 <= NWIN:
            avail_n = counts[w + 1] - taken_from_half[w + 1]
            t1 = min(avail_n, cap)
            if t1 > 0:
                s = starts[w + 1] + taken_from_half[w + 1]
                assign_w[s:s + t1] = w
                taken_from_half[w + 1] += t1
                cap -= t1
        win_n[w] = Q[w] - cap
    assert taken_from_half[:NWIN + 1].sum() == n, "assignment incomplete"
    return assign_w, win_n


def _make_schedule(images, boxes):
    B = images.shape[0]
    per_core = []
    hc = np.zeros((B, NWIN + 1), np.int64)
    for c in range(B):
        gu, w8 = _host_point_data(boxes[c])
        order = np.argsort(gu, kind='stable')
        gus = gu[order]
        per_core.append((gus, order, w8))
        hcounts = np.bincount(gus >> 14, minlength=NWIN + 1)
        hc[c] = hcounts

    Q = _window_quotas(hc)
    pk = Q                                        # idx slots per window
    total = int(pk.sum())
    L = total                                     # multiple of P already
    ncols = L // P
    # chunk column widths: full TGT chunks + a smaller tail chunk
    ccols = [MCOL] * (ncols // MCOL)
    if ncols % MCOL:
        ccols.append(ncols % MCOL)
    cslot = np.cumsum([0] + [cc * P for cc in ccols])

    segments = [[] for _ in ccols]
    off = 0
    for k in range(NWIN):
        remaining = int(pk[k])
        while remaining > 0:
            ch = int(np.searchsorted(cslot, off, 'right')) - 1
            room = int(cslot[ch + 1]) - off
            take = min(room, remaining)
            segments[ch].append((k, off - int(cslot[ch]), take))
            off += take
            remaining -= take

    in_maps = []
    unsort_rows = []
    base_of = np.cumsum(np.concatenate([[0], pk[:-1]]))
    for c in range(B):
        gus, order, w8 = per_core[c]
        assign_w, win_n = _assign(gus, Q)
        flat = np.zeros(L, np.int16)
        flatW = np.zeros((L, 8), NPBF16)
        pos_of = np.empty(NPT, np.int64)
        # place points window by window (they are contiguous in sorted order
        # except interleaving at half boundaries; gather positions per window)
        for k in range(NWIN):
            pts_sorted_idx = np.nonzero(assign_w == k)[0]
            npts = len(pts_sorted_idx)
            assert npts <= pk[k], (k, npts, pk[k])
            sl = slice(int(base_of[k]), int(base_of[k]) + npts)
            rel = gus[pts_sorted_idx] - k * HWIN
            assert (rel >= 0).all() and (rel < WIN).all()
            flat[sl] = rel.astype(np.int16)
            pid = order[pts_sorted_idx]
            flatW[sl] = w8[pid]
            pos_of[pid] = np.arange(sl.start, sl.start + npts)

        wrap = flat.reshape(L // 16, 16).T
        idxw = np.tile(wrap, (8, 1)).copy()
        wts = np.ascontiguousarray(
            flatW.reshape(L // P, P, 8).transpose(1, 0, 2).reshape(P, -1))
        cp = _repack4(images[c])
        in_maps.append({"cp": cp, "idx": idxw, "wts": wts})

        q = pos_of
        ch = np.searchsorted(cslot, q, 'right') - 1
        cc_of = np.asarray(ccols)[ch]
        unsort_rows.append(cslot[ch] + (q % P) * cc_of + (q - cslot[ch]) // P)

    return in_maps, segments, unsort_rows, L


def _build_nc(segments, L, num_devices=8, repeat=1):
    nc = bacc.Bacc("TRN2", target_bir_lowering=False, debug=False,
                   num_devices=num_devices, num_swdge_queues=NQ)
    cp_d = nc.dram_tensor("cp", [4 * QUNITS * UNIT], BF16,
                          kind="ExternalInput")
    idx_d = nc.dram_tensor("idx", [P, L // 16], I16, kind="ExternalInput")
    wts_d = nc.dram_tensor("wts", [P, (L // P) * 8], BF16,
                           kind="ExternalInput")
    out_d = nc.dram_tensor("out", [L, C], BF16, kind="ExternalOutput")

    nchunks = len(segments)
    ccols = [sum(cnt for _, _, cnt in seg) // P for seg in segments]
    cslot = [0]
    for cc in ccols:
        cslot.append(cslot[-1] + cc * P)
    assert cslot[-1] == L

    def win_in_ap(k):
        return AP(cp_d, k * HWIN * UNIT, [[UNIT, WIN], [1, UNIT]])

    with tile.TileContext(nc) as tc:
        with tc.tile_pool(name="persist", bufs=1) as pp:
            idx = pp.tile([P, L // 16], I16)
            wts = pp.tile([P, (L // P) * 8], BF16)
            nc.sync.dma_start(idx[:], idx_d.ap())
            nc.sync.dma_start(wts[:], wts_d.ap())
            wtsv = wts[:].rearrange("p (m s) -> p m s", s=8)

            with tc.tile_pool(name="work", bufs=3) as wp:

                for _ in range(repeat):
                    for ci in range(nchunks):
                        cc = ccols[ci]
                        X0 = wp.tile([P, CHUNK], BF16, tag="X0")
                        G0 = (X0[:, :cc * UNIT]
                              .rearrange("p (m e) -> p m e", e=UNIT))
                        if not AB_SKIP_GATHER:
                            for (k, s0, cnt) in segments[ci]:
                                gcol = (cslot[ci] + s0) // 16
                                m0 = s0 // P
                                mw = cnt // P
                                jobs = []
                                if mw >= NQ:
                                    mh = mw // NQ
                                    off = 0
                                    for j in range(NQ):
                                        mj = (mw - off if j == NQ - 1
                                              else mh)
                                        jobs.append(
                                            (j, m0 + off, mj,
                                             gcol + (off * P) // 16))
                                        off += mj
                                else:
                                    for j in range(mw):
                                        jobs.append((j % NQ, m0 + j, 1,
                                                     gcol + (j * P) // 16))
                                for (q, mm0, mmw, gc) in jobs:
                                    nc.gpsimd.dma_gather(
                                        out_ap=G0[:, mm0:mm0 + mmw, :],
                                        in_ap=win_in_ap(k),
                                        idxs_ap=idx[:, gc:gc
                                                    + (mmw * P) // 16],
                                        num_idxs=mmw * P,
                                        num_idxs_reg=mmw * P,
                                        elem_size=UNIT,
                                        elem_step=UNIT,
                                        single_packet=False,
                                        queue_num=q,
                                    )

                        res = wp.tile([P, MCOL * C], BF16, tag="res")
                        tmp = wp.tile([P, MCOL * C], BF16, tag="tmp")
                        r4 = (res[:, :cc * C]
                              .rearrange("p (m c2 two) -> p m c2 two",
                                         m=cc, two=2))
                        t4 = (tmp[:, :cc * C]
                              .rearrange("p (m c2 two) -> p m c2 two",
                                         m=cc, two=2))
                        colbase = cslot[ci] // P

                        def wb(s):
                            return (wtsv[:, colbase:colbase + cc,
                                         2 * s:2 * s + 2]
                                    .unsqueeze(2)
                                    .to_broadcast([P, cc, C // 2, 2]))

                        srcs = list(range(4))
                        if AB_SKIP_BLEND:
                            srcs = srcs[:1]
                        first = True
                        for s in srcs:
                            sl = (G0[:, :, s * C:(s + 1) * C]
                                  .rearrange("p m (c2 two) -> p m c2 two",
                                             two=2))
                            if first:
                                nc.vector.tensor_tensor(r4, sl, wb(s),
                                                        op=OP.mult)
                                first = False
                            else:
                                nc.vector.tensor_tensor(t4, sl, wb(s),
                                                        op=OP.mult)
                                nc.vector.tensor_tensor(r4, r4, t4,
                                                        op=OP.add)
                        if not AB_SKIP_OUT:
                            outap = AP(out_d, cslot[ci] * C,
                                       [[cc * C, P], [1, cc * C]])
                            nc.sync.dma_start(outap, res[:, :cc * C])

    nc.compile()
    return nc


_NC_CACHE = {}


def kernel(images, boxes):
    images = np.ascontiguousarray(np.asarray(images, dtype=np.float32))
    boxes = np.ascontiguousarray(np.asarray(boxes, dtype=np.float32))
    B = images.shape[0]

    in_maps, segments, unsort_rows, L = _make_schedule(images, boxes)

    key = (B, L, tuple(tuple(s) for cs in segments for s in cs))
    nc = _NC_CACHE.get(key)
    if nc is None:
        nc = _build_nc(segments, L, num_devices=B)
        _NC_CACHE.clear()
        _NC_CACHE[key] = nc

    from concourse import bass_utils
    res = bass_utils.run_bass_kernel_spmd(nc, in_maps, core_ids=list(range(B)))

    outs = []
    for c in range(B):
        scratch = res.results[c]["out"]
        outs.append(scratch[unsort_rows[c]].astype(np.float32)
                    .reshape(NB, CROP, CROP, C))
    return np.concatenate(outs, axis=0)
